# revision 54
# baseline (speedup 1.0000x reference)
"""Fused multi-LoRA linear layer on 8 TRN2 NeuronCores.

out = x @ W.T + b + scale * mask(x @ A_all^T) @ B_flat

Sharding: data-parallel over the token dim N (32768 -> 8 x 4096).
Weights (W, A_all, B_all, b) are replicated; each core computes its token
shard fully, so no collectives are needed.

Device-side layout: the kernel computes out^T [d_out, tokens] so that the
bias is a per-partition scalar (fused into the PSUM->SBUF eviction on the
Scalar engine) and neither x nor the output needs an on-chip transpose.
All streamed inputs are laid out partition-major on the host so every DMA
is a contiguous-per-partition block transfer.

Mixed precision: the last KF=6 of 16 k-tiles of the main matmul run in
fp8e4 (e4m3) with DoubleRow perf mode (2 contraction rows per PE
cell-cycle); the other 10 k-tiles and the LoRA path stay bf16.
fp8 operands are pre-scaled on the host (x*16, W*256) so their PSUM
contribution lands at 4096x; the bf16 x tiles are pre-scaled by 4096
(exact, power of two) so the whole PSUM accumulates at 4096x, and the
eviction folds the 2^-12 rescale into the scalar-engine activation.
Measured rel err vs the f32 reference: 1.945e-2 (gate: 2e-2).
"""

import numpy as np
import ml_dtypes

# Problem constants (hardcoded per harness contract).
N, D_IN, D_OUT, L, R = 32768, 2048, 2048, 8, 16
SCALE = 32.0 / 16.0
M_CORES = 8
NS = N // M_CORES  # 4096 tokens per core
P = 128
KT = D_IN // P  # 16 k-tiles
KF = 6  # fp8 k-tiles of the main matmul (the last KF)
KB = KT - KF  # bf16 k-tiles
OI = D_OUT // P  # 16 output row-chunks of 128
TW = 512  # token tile width (moving free dim)
TC = NS // TW  # 8 token chunks per core
LR = L * R  # 128
WG = 4  # W column groups (bf16 part)
WGC = D_OUT // WG  # 512 columns per group
SX = 16.0  # fp8 x scale
SW = 256.0  # fp8 W / A scale
PSCALE = SX * SW  # 4096: PSUM runs at this scale

_BF16 = ml_dtypes.bfloat16
_F8 = ml_dtypes.float8_e4m3

_CACHE = {}

LAST_EXEC_TIME_NS = None


def _build():
    import concourse.bass as bass  # noqa: F401
    import concourse.tile as tile
    from concourse import bacc, mybir
    from contextlib import ExitStack

    bf16 = mybir.dt.bfloat16
    f8 = mybir.dt.float8e4
    f32 = mybir.dt.float32
    DR = mybir.MatmulPerfMode.DoubleRow

    nc = bacc.Bacc(
        "TRN2",
        target_bir_lowering=False,
        debug=False,
        num_devices=M_CORES,
    )

    # Host-prepared, partition-major layouts (see kernel()):
    #   xT  [TC, P, KB, TW] : xT[t, p, k, j] = 4096*x[t*TW+j, k*P+p]   (bf16)
    #   xF  [TC, P, KF, TW] : xF[t, p, k, j] = e4m3(16*x[t*TW+j, (KB+k)*P+p])
    #   wT  [WG, P, KB, WGC]: wT[g, p, k, o] = W[g*WGC+o, k*P+p]       (bf16)
    #   wF  [P, KF, D_OUT]  : wF[p, k, o] = e4m3(256*W[o, (KB+k)*P+p])
    #   aT  [P, KB, LR]     : aT[p, k, c] = A_flat[c, k*P+p]           (bf16)
    #   aF  [P, KF, LR]     : aF[p, k, c] = e4m3(256*A_flat[c, (KB+k)*P+p])
    #   bF  [P, D_OUT]      : bF[c, o] = B_all[c//R, o, c%R]           (bf16)
    #   mT  [TC, P, TW]     : one-hot adapter mask * SCALE             (bf16)
    #   bias [P, OI]        : bias[p, oi] = b[oi*P+p]                  (f32)
    xT = nc.dram_tensor("xT", [TC, P, KB, TW], bf16, kind="ExternalInput").ap()
    xF = nc.dram_tensor("xF", [TC, P, KF, TW], f8, kind="ExternalInput").ap()
    wT = nc.dram_tensor("wT", [WG, P, KB, WGC], bf16, kind="ExternalInput").ap()
    wF = nc.dram_tensor("wF", [P, KF, D_OUT], f8, kind="ExternalInput").ap()
    aT = nc.dram_tensor("aT", [P, KB, LR], bf16, kind="ExternalInput").ap()
    aF = nc.dram_tensor("aF", [P, KF, LR], f8, kind="ExternalInput").ap()
    bF = nc.dram_tensor("bF", [P, D_OUT], bf16, kind="ExternalInput").ap()
    bias = nc.dram_tensor("bias", [P, OI], f32, kind="ExternalInput").ap()
    mT = nc.dram_tensor("mT", [TC, P, TW], bf16, kind="ExternalInput").ap()
    # bf16 output halves the out-DMA traffic; the host upcasts. Costs
    # ~0.23% RMS in quadrature with the fp8 error - still under the gate.
    outT = nc.dram_tensor("outT", [D_OUT, NS], bf16, kind="ExternalOutput").ap()

    from concourse.tile_rust import add_dep_helper

    with tile.TileContext(nc) as tc, ExitStack() as ctx:
        warm_pool = ctx.enter_context(tc.tile_pool(name="warm", bufs=1))
        wt_pool = ctx.enter_context(tc.tile_pool(name="wt", bufs=WG))
        wf_pool = ctx.enter_context(tc.tile_pool(name="wf", bufs=1))
        af_pool = ctx.enter_context(tc.tile_pool(name="af", bufs=1))
        bf_pool = ctx.enter_context(tc.tile_pool(name="bfp", bufs=1))
        bias_pool = ctx.enter_context(tc.tile_pool(name="bias", bufs=1))
        mask_pool = ctx.enter_context(tc.tile_pool(name="mask", bufs=1))
        x_pool = ctx.enter_context(tc.tile_pool(name="x", bufs=2))
        x8_pool = ctx.enter_context(tc.tile_pool(name="x8", bufs=2))
        u_pool = ctx.enter_context(tc.tile_pool(name="u", bufs=2))
        o_pool = ctx.enter_context(tc.tile_pool(name="o", bufs=8))
        pw_pool = ctx.enter_context(tc.tile_pool(name="pw", bufs=1, space="PSUM"))
        pu_pool = ctx.enter_context(tc.tile_pool(name="pu", bufs=2, space="PSUM"))
        po_pool = ctx.enter_context(tc.tile_pool(name="po", bufs=5, space="PSUM"))

        # Short PE warmup: real matmuls are ready almost immediately (the
        # first x quarter lands before the engine preamble finishes), so the
        # warmup only needs to absorb a slice of the HAM cold-clock window.
        warm = warm_pool.tile([P, P], bf16)
        nc.vector.memset(warm[:], 0.0)
        pw = pw_pool.tile([P, P], mybir.dt.float32)
        for _ in range(16):
            nc.tensor.matmul(pw[:], warm[:], warm[:], start=True, stop=True)

        # A tiles ride the front of the scalar ring (idle until the gated W
        # stream starts); the first x chunk is the sync-ring critical path
        # (issued inside the t=0 loop iteration below). mask/bias go on the
        # gpsimd ring - they are not needed until the first up-projection.
        at = af_pool.tile([P, KB, LR], bf16)
        nc.scalar.dma_start(at[:], aT[:, :, :])
        af8 = af_pool.tile([P, KF, LR], f8)
        nc.scalar.dma_start(af8[:], aF[:, :, :])
        bias_t = bias_pool.tile([P, OI], f32)
        nc.gpsimd.dma_start(bias_t[:], bias[:, :])
        mask_t = mask_pool.tile([P, TC, TW], bf16)
        nc.gpsimd.dma_start(mask_t[:], mT.rearrange("t p j -> p t j"))

        # Big W load + B_flat stream on the scalar HWDGE ring, gated behind
        # the first x quarter so the critical-path x DMA is not starved at
        # kickoff. Order: g0 (needed by oi=0), then the small bF/wF (needed
        # by every oi's tail), then g1-g3.
        wts = [
            wt_pool.tile([P, KB, WGC], bf16, name="wt_g") for _ in range(WG)
        ]
        wf_t = wf_pool.tile([P, KF, D_OUT], f8)
        bf_t = bf_pool.tile([P, D_OUT], bf16)
        wg0_dma = nc.scalar.dma_start(wts[0][:], wT[0])
        nc.scalar.dma_start(bf_t[:], bF[:, :])
        for g in range(1, WG):
            nc.scalar.dma_start(wts[g][:], wT[g])

        # Chunk 0 streams in k-tile slices so the down-projection can start
        # on tile 0 before the whole chunk lands; W yields HBM bandwidth
        # until the first two slices are in.
        xc = x_pool.tile([P, KB, TW], bf16, name="xc", bufs=2)
        for lo, hi in ((0, 4), (4, 8), (8, KB)):
            xq_dma = nc.sync.dma_start(xc[:, lo:hi, :], xT[0, :, lo:hi, :])
            if hi <= 8:
                add_dep_helper(
                    wg0_dma.ins, xq_dma.ins, sync=True,
                    reason="critical path first",
                )
        x8 = x8_pool.tile([P, KF, TW], f8)
        nc.sync.dma_start(x8[:], xF[0])
        # wF rides the sync ring behind chunk 0's x (the scalar ring is busy
        # with g0/bF until past the point oi=0 needs it).
        nc.sync.dma_start(wf_t[:], wF[:, :, :])

        for t in range(TC):
            # Prefetch the next chunk's x now, ahead of this chunk's output
            # DMAs on the in-order sync ring - otherwise the next
            # down-projection waits for 4 MB of outs to drain.
            if t + 1 < TC:
                xc_next = x_pool.tile([P, KB, TW], bf16, name="xc", bufs=2)
                nc.sync.dma_start(xc_next[:], xT[t + 1])
                x8_next = x8_pool.tile([P, KF, TW], f8, name="x8", bufs=2)
                nc.sync.dma_start(x8_next[:], xF[t + 1])

            # LoRA down-projection: u^T[c, tok] for all adapters at once,
            # at 4096x scale (x carries the 4096). The last KF k-tiles ride
            # the fp8 DoubleRow path like the main matmul.
            pu = pu_pool.tile([P, TW], mybir.dt.float32)
            for k in range(KB):
                nc.tensor.matmul(
                    pu[:], at[:, k, :], xc[:, k, :], start=(k == 0), stop=False
                )
            for i in range(KF // 2):
                nc.tensor.matmul(
                    pu[:],
                    af8[:, 2 * i : 2 * i + 2, :],
                    x8[:, 2 * i : 2 * i + 2, :],
                    start=False,
                    stop=(i == KF // 2 - 1),
                    perf_mode=DR,
                )
            # Mask-select adapters + apply scale (mask carries the scale;
            # pu and the main PSUM share the 4096x scale so it cancels).
            um = u_pool.tile([P, TW], bf16)
            nc.vector.tensor_tensor(
                um[:], pu[:], mask_t[:, t, :], op=mybir.AluOpType.mult
            )

            for oi in range(OI):
                wt_g = wts[oi // WG]
                loc = (oi % WG) * P
                po = po_pool.tile([P, TW], mybir.dt.float32)
                for k in range(KB):
                    nc.tensor.matmul(
                        po[:],
                        wt_g[:, k, loc : loc + P],
                        xc[:, k, :],
                        start=(k == 0),
                        stop=False,
                    )
                # fp8 DoubleRow k-tiles (the last KF of the contraction).
                for i in range(KF // 2):
                    nc.tensor.matmul(
                        po[:],
                        wf_t[:, 2 * i : 2 * i + 2, oi * P : (oi + 1) * P],
                        x8[:, 2 * i : 2 * i + 2, :],
                        start=False,
                        stop=False,
                        perf_mode=DR,
                    )
                # LoRA up-projection accumulates into the same PSUM bank.
                nc.tensor.matmul(
                    po[:], bf_t[:, oi * P : (oi + 1) * P], um[:], start=False, stop=True
                )
                ot = o_pool.tile([P, TW], bf16)
                # Eviction rescales PSUM (4096x) and adds the bias.
                nc.scalar.activation(
                    ot[:],
                    po[:],
                    mybir.ActivationFunctionType.Identity,
                    bias=bias_t[:, oi : oi + 1],
                    scale=1.0 / PSCALE,
                )
                nc.sync.dma_start(
                    outT[oi * P : (oi + 1) * P, t * TW : (t + 1) * TW], ot[:]
                )
            if t + 1 < TC:
                xc, x8 = xc_next, x8_next

    nc.compile()
    return nc


def _get_nc():
    if "nc" not in _CACHE:
        _CACHE["nc"] = _build()
    return _CACHE["nc"]


def _install_trace_shim():
    """This image's antenv lacks axon_hooks; register the NTFF profile hook
    ourselves so run_bass_kernel_spmd(trace=True) can capture exec_time_ns."""
    import sys
    import types

    if "antenv.axon_hooks" in sys.modules:
        return
    import antenv

    mod = types.ModuleType("antenv.axon_hooks")
    state = {"hook": None}
    mod.set_axon_ntff_profile_hook = lambda h: state.__setitem__("hook", h)
    mod.get_axon_ntff_profile_hook = lambda: state["hook"]
    sys.modules["antenv.axon_hooks"] = mod
    antenv.axon_hooks = mod

    from trn_agent_boot.trn_boot import _ntff_profile_via_ctypes

    mod.set_axon_ntff_profile_hook(
        _ntff_profile_via_ctypes("/opt/axon/libaxon_pjrt.so")
    )

    # No S3 in this container; keep artifacts local.
    import concourse.bass_utils as bu

    bu.upload_artifacts = lambda tmpdir: f"local://{tmpdir}"


def kernel(x, W, b, A_all, B_all, lora_idx, _trace=False):
    global LAST_EXEC_TIME_NS
    from concourse.bass_utils import run_bass_kernel_spmd

    if _trace:
        try:
            _install_trace_shim()
        except Exception as e:  # degrade to untraced run
            print(f"trace shim failed ({e!r}); running untraced")
            _trace = False

    x = np.asarray(x, dtype=np.float32)
    W = np.asarray(W, dtype=np.float32)
    b = np.asarray(b, dtype=np.float32)
    A_all = np.asarray(A_all, dtype=np.float32)
    B_all = np.asarray(B_all, dtype=np.float32)
    lora_idx = np.asarray(lora_idx, dtype=np.int32)

    # Host-side weight reformat (replicated across cores), partition-major.
    # wT[g, p, k, o] = W[g*WGC+o, k*P+p] for the KB bf16 k-tiles.
    wT_np = np.ascontiguousarray(
        W.astype(_BF16).reshape(WG, WGC, KT, P)[:, :, :KB].transpose(0, 3, 2, 1)
    )
    # wF[p, k, o] = e4m3(SW * W[o, (KB+k)*P+p])
    wF_np = np.ascontiguousarray(
        (W[:, KB * P :] * SW).astype(_F8).reshape(D_OUT, KF, P).transpose(2, 1, 0)
    )
    # aT[p, k, c] = A_flat[c, k*P+p] (bf16 k-tiles); aF: fp8 tail tiles
    A_kt = A_all.reshape(LR, KT, P)
    aT_np = np.ascontiguousarray(A_kt[:, :KB].astype(_BF16).transpose(2, 1, 0))
    aF_np = np.ascontiguousarray(
        (A_kt[:, KB:] * SW).astype(_F8).transpose(2, 1, 0)
    )
    # bF[c, o] = B_all[c//R, o, c%R]
    bF_np = np.ascontiguousarray(B_all.transpose(0, 2, 1)).reshape(LR, D_OUT).astype(
        _BF16
    )
    bias_np = np.ascontiguousarray(b.reshape(OI, P).T).astype(np.float32)

    xb = (x * PSCALE).astype(_BF16)  # exact power-of-2 scale
    x8 = (x[:, KB * P :] * SX).astype(_F8)
    adapters = (np.arange(LR, dtype=np.int32) // R)[:, None]  # [LR, 1]

    in_maps = []
    for i in range(M_CORES):
        s = slice(i * NS, (i + 1) * NS)
        # xT[t, p, k, j] = PSCALE * x[i*NS + t*TW + j, k*P + p], k < KB
        xT_i = np.ascontiguousarray(
            xb[s].reshape(TC, TW, KT, P)[:, :, :KB].transpose(0, 3, 2, 1)
        )
        xF_i = np.ascontiguousarray(
            x8[s].reshape(TC, TW, KF, P).transpose(0, 3, 2, 1)
        )
        idx = lora_idx[s]
        mfull = (adapters == idx[None, :]).astype(np.float32) * SCALE  # [LR, NS]
        mT_i = np.ascontiguousarray(
            mfull.astype(_BF16).reshape(LR, TC, TW).transpose(1, 0, 2)
        )
        in_maps.append(
            {
                "xT": xT_i,
                "xF": xF_i,
                "wT": wT_np,
                "wF": wF_np,
                "aT": aT_np,
                "aF": aF_np,
                "bF": bF_np,
                "bias": bias_np,
                "mT": mT_i,
            }
        )

    nc = _get_nc()
    res = run_bass_kernel_spmd(
        nc, in_maps, core_ids=list(range(M_CORES)), trace=_trace
    )
    LAST_EXEC_TIME_NS = res.exec_time_ns

    out = np.empty((N, D_OUT), dtype=np.float32)
    for i in range(M_CORES):
        out[i * NS : (i + 1) * NS] = res.results[i]["outT"].T.astype(np.float32)
    return out


# revision 57
# speedup vs baseline: 1.0005x; 1.0005x over previous
"""Fused multi-LoRA linear layer on 8 TRN2 NeuronCores.

out = x @ W.T + b + scale * mask(x @ A_all^T) @ B_flat

Sharding: data-parallel over the token dim N (32768 -> 8 x 4096).
Weights (W, A_all, B_all, b) are replicated; each core computes its token
shard fully, so no collectives are needed.

Device-side layout: the kernel computes out^T [d_out, tokens] so that the
bias is a per-partition scalar (fused into the PSUM->SBUF eviction on the
Scalar engine) and neither x nor the output needs an on-chip transpose.
All streamed inputs are laid out partition-major on the host so every DMA
is a contiguous-per-partition block transfer.

Mixed precision: the last KF=6 of 16 k-tiles of the main matmul run in
fp8e4 (e4m3) with DoubleRow perf mode (2 contraction rows per PE
cell-cycle); the other 10 k-tiles and the LoRA path stay bf16.
fp8 operands are pre-scaled on the host (x*16, W*256) so their PSUM
contribution lands at 4096x; the bf16 x tiles are pre-scaled by 4096
(exact, power of two) so the whole PSUM accumulates at 4096x, and the
eviction folds the 2^-12 rescale into the scalar-engine activation.
Measured rel err vs the f32 reference: 1.945e-2 (gate: 2e-2).
"""

import numpy as np
import ml_dtypes

# Problem constants (hardcoded per harness contract).
N, D_IN, D_OUT, L, R = 32768, 2048, 2048, 8, 16
SCALE = 32.0 / 16.0
M_CORES = 8
NS = N // M_CORES  # 4096 tokens per core
P = 128
KT = D_IN // P  # 16 k-tiles
KF = 6  # fp8 k-tiles of the main matmul (the last KF)
KB = KT - KF  # bf16 k-tiles
OI = D_OUT // P  # 16 output row-chunks of 128
TW = 512  # token tile width (moving free dim)
TC = NS // TW  # 8 token chunks per core
LR = L * R  # 128
WG = 4  # W column groups (bf16 part)
WGC = D_OUT // WG  # 512 columns per group
SX = 16.0  # fp8 x scale
SW = 256.0  # fp8 W / A scale
PSCALE = SX * SW  # 4096: PSUM runs at this scale

_BF16 = ml_dtypes.bfloat16
_F8 = ml_dtypes.float8_e4m3

_CACHE = {}

LAST_EXEC_TIME_NS = None


def _build():
    import concourse.bass as bass  # noqa: F401
    import concourse.tile as tile
    from concourse import bacc, mybir
    from contextlib import ExitStack

    bf16 = mybir.dt.bfloat16
    f8 = mybir.dt.float8e4
    f32 = mybir.dt.float32
    DR = mybir.MatmulPerfMode.DoubleRow

    nc = bacc.Bacc(
        "TRN2",
        target_bir_lowering=False,
        debug=False,
        num_devices=M_CORES,
    )

    # Host-prepared, partition-major layouts (see kernel()):
    #   xT  [TC, P, KB, TW] : xT[t, p, k, j] = 4096*x[t*TW+j, k*P+p]   (bf16)
    #   xF  [TC, P, KF, TW] : xF[t, p, k, j] = e4m3(16*x[t*TW+j, (KB+k)*P+p])
    #   wT  [WG, P, KB, WGC]: wT[g, p, k, o] = W[g*WGC+o, k*P+p]       (bf16)
    #   wF  [P, KF, D_OUT]  : wF[p, k, o] = e4m3(256*W[o, (KB+k)*P+p])
    #   aT  [P, KB, LR]     : aT[p, k, c] = A_flat[c, k*P+p]           (bf16)
    #   aF  [P, KF, LR]     : aF[p, k, c] = e4m3(256*A_flat[c, (KB+k)*P+p])
    #   bF  [P, D_OUT]      : bF[c, o] = B_all[c//R, o, c%R]           (bf16)
    #   mT  [TC, P, TW]     : one-hot adapter mask * SCALE             (bf16)
    #   bias [P, OI]        : bias[p, oi] = b[oi*P+p]                  (f32)
    xT = nc.dram_tensor("xT", [TC, P, KB, TW], bf16, kind="ExternalInput").ap()
    xF = nc.dram_tensor("xF", [TC, P, KF, TW], f8, kind="ExternalInput").ap()
    wT = nc.dram_tensor("wT", [WG, P, KB, WGC], bf16, kind="ExternalInput").ap()
    wF = nc.dram_tensor("wF", [P, KF, D_OUT], f8, kind="ExternalInput").ap()
    aT = nc.dram_tensor("aT", [P, KB, LR], bf16, kind="ExternalInput").ap()
    aF = nc.dram_tensor("aF", [P, KF, LR], f8, kind="ExternalInput").ap()
    bF = nc.dram_tensor("bF", [P, D_OUT], bf16, kind="ExternalInput").ap()
    bias = nc.dram_tensor("bias", [P, OI], f32, kind="ExternalInput").ap()
    mT = nc.dram_tensor("mT", [TC, P, TW], bf16, kind="ExternalInput").ap()
    outT = nc.dram_tensor("outT", [D_OUT, NS], f32, kind="ExternalOutput").ap()

    from concourse.tile_rust import add_dep_helper

    with tile.TileContext(nc) as tc, ExitStack() as ctx:
        warm_pool = ctx.enter_context(tc.tile_pool(name="warm", bufs=1))
        wt_pool = ctx.enter_context(tc.tile_pool(name="wt", bufs=WG))
        wf_pool = ctx.enter_context(tc.tile_pool(name="wf", bufs=1))
        af_pool = ctx.enter_context(tc.tile_pool(name="af", bufs=1))
        bf_pool = ctx.enter_context(tc.tile_pool(name="bfp", bufs=1))
        bias_pool = ctx.enter_context(tc.tile_pool(name="bias", bufs=1))
        mask_pool = ctx.enter_context(tc.tile_pool(name="mask", bufs=1))
        x_pool = ctx.enter_context(tc.tile_pool(name="x", bufs=2))
        x8_pool = ctx.enter_context(tc.tile_pool(name="x8", bufs=2))
        u_pool = ctx.enter_context(tc.tile_pool(name="u", bufs=2))
        o_pool = ctx.enter_context(tc.tile_pool(name="o", bufs=8))
        pw_pool = ctx.enter_context(tc.tile_pool(name="pw", bufs=1, space="PSUM"))
        pu_pool = ctx.enter_context(tc.tile_pool(name="pu", bufs=2, space="PSUM"))
        po_pool = ctx.enter_context(tc.tile_pool(name="po", bufs=5, space="PSUM"))

        # Short PE warmup: real matmuls are ready almost immediately (the
        # first x quarter lands before the engine preamble finishes), so the
        # warmup only needs to absorb a slice of the HAM cold-clock window.
        warm = warm_pool.tile([P, P], bf16)
        nc.vector.memset(warm[:], 0.0)
        pw = pw_pool.tile([P, P], mybir.dt.float32)
        for _ in range(72):
            nc.tensor.matmul(pw[:], warm[:], warm[:], start=True, stop=True)

        def keep_warm(n):
            # Dummy matmuls sized just under a known DMA-stall window: they
            # keep the PE busy so the HAM clock gate stays at 8/8 instead of
            # re-throttling to 1.2 GHz across the >3.4us idle window.
            for _ in range(n):
                nc.tensor.matmul(pw[:], warm[:], warm[:], start=True, stop=True)

        # A tiles ride the front of the scalar ring (idle until the gated W
        # stream starts); the first x chunk is the sync-ring critical path
        # (issued inside the t=0 loop iteration below). mask/bias go on the
        # gpsimd ring - they are not needed until the first up-projection.
        at = af_pool.tile([P, KB, LR], bf16)
        nc.scalar.dma_start(at[:], aT[:, :, :])
        af8 = af_pool.tile([P, KF, LR], f8)
        nc.scalar.dma_start(af8[:], aF[:, :, :])
        bias_t = bias_pool.tile([P, OI], f32)
        nc.gpsimd.dma_start(bias_t[:], bias[:, :])
        mask_t = mask_pool.tile([P, TC, TW], bf16)
        nc.gpsimd.dma_start(mask_t[:], mT.rearrange("t p j -> p t j"))

        # Big W load + B_flat stream on the scalar HWDGE ring, gated behind
        # the first x quarter so the critical-path x DMA is not starved at
        # kickoff. Order: g0 (needed by oi=0), then the small bF/wF (needed
        # by every oi's tail), then g1-g3.
        wts = [
            wt_pool.tile([P, KB, WGC], bf16, name="wt_g") for _ in range(WG)
        ]
        wf_t = wf_pool.tile([P, KF, D_OUT], f8)
        bf_t = bf_pool.tile([P, D_OUT], bf16)
        wg0_dma = nc.scalar.dma_start(wts[0][:], wT[0])
        nc.scalar.dma_start(bf_t[:], bF[:, :])
        for g in range(1, WG):
            nc.scalar.dma_start(wts[g][:], wT[g])

        # Chunk 0 streams in k-tile slices so the down-projection can start
        # on tile 0 before the whole chunk lands; W yields HBM bandwidth
        # until the first two slices are in.
        xc = x_pool.tile([P, KB, TW], bf16, name="xc", bufs=2)
        for lo, hi in ((0, 4), (4, 8), (8, KB)):
            xq_dma = nc.sync.dma_start(xc[:, lo:hi, :], xT[0, :, lo:hi, :])
            if hi <= 8:
                add_dep_helper(
                    wg0_dma.ins, xq_dma.ins, sync=True,
                    reason="critical path first",
                )
        x8 = x8_pool.tile([P, KF, TW], f8)
        nc.sync.dma_start(x8[:], xF[0])
        # wF rides the sync ring behind chunk 0's x (the scalar ring is busy
        # with g0/bF until past the point oi=0 needs it).
        nc.sync.dma_start(wf_t[:], wF[:, :, :])

        for t in range(TC):
            # Prefetch the next chunk's x now, ahead of this chunk's output
            # DMAs on the in-order sync ring - otherwise the next
            # down-projection waits for 4 MB of outs to drain.
            if t + 1 < TC:
                xc_next = x_pool.tile([P, KB, TW], bf16, name="xc", bufs=2)
                nc.sync.dma_start(xc_next[:], xT[t + 1])
                x8_next = x8_pool.tile([P, KF, TW], f8, name="x8", bufs=2)
                nc.sync.dma_start(x8_next[:], xF[t + 1])

            # LoRA down-projection: u^T[c, tok] for all adapters at once,
            # at 4096x scale (x carries the 4096). The last KF k-tiles ride
            # the fp8 DoubleRow path like the main matmul.
            pu = pu_pool.tile([P, TW], mybir.dt.float32)
            for k in range(KB):
                nc.tensor.matmul(
                    pu[:], at[:, k, :], xc[:, k, :], start=(k == 0), stop=False
                )
                if t == 0 and k in (3, 7):
                    # Bridge the measured chunk-0 x-slice DMA waits (~4-5us
                    # each) so the HAM stays warm through them.
                    keep_warm(60 if k == 3 else 75)
            if t == 0:
                keep_warm(55)  # bridge the x8/wF wait before the DR tail
            for i in range(KF // 2):
                nc.tensor.matmul(
                    pu[:],
                    af8[:, 2 * i : 2 * i + 2, :],
                    x8[:, 2 * i : 2 * i + 2, :],
                    start=False,
                    stop=(i == KF // 2 - 1),
                    perf_mode=DR,
                )
            # Mask-select adapters + apply scale (mask carries the scale;
            # pu and the main PSUM share the 4096x scale so it cancels).
            um = u_pool.tile([P, TW], bf16)
            nc.vector.tensor_tensor(
                um[:], pu[:], mask_t[:, t, :], op=mybir.AluOpType.mult
            )

            for oi in range(OI):
                wt_g = wts[oi // WG]
                loc = (oi % WG) * P
                po = po_pool.tile([P, TW], mybir.dt.float32)
                for k in range(KB):
                    nc.tensor.matmul(
                        po[:],
                        wt_g[:, k, loc : loc + P],
                        xc[:, k, :],
                        start=(k == 0),
                        stop=False,
                    )
                # fp8 DoubleRow k-tiles (the last KF of the contraction).
                for i in range(KF // 2):
                    nc.tensor.matmul(
                        po[:],
                        wf_t[:, 2 * i : 2 * i + 2, oi * P : (oi + 1) * P],
                        x8[:, 2 * i : 2 * i + 2, :],
                        start=False,
                        stop=False,
                        perf_mode=DR,
                    )
                # LoRA up-projection accumulates into the same PSUM bank.
                nc.tensor.matmul(
                    po[:], bf_t[:, oi * P : (oi + 1) * P], um[:], start=False, stop=True
                )
                ot = o_pool.tile([P, TW], mybir.dt.float32)
                # Eviction rescales PSUM (4096x) and adds the bias.
                nc.scalar.activation(
                    ot[:],
                    po[:],
                    mybir.ActivationFunctionType.Identity,
                    bias=bias_t[:, oi : oi + 1],
                    scale=1.0 / PSCALE,
                )
                nc.sync.dma_start(
                    outT[oi * P : (oi + 1) * P, t * TW : (t + 1) * TW], ot[:]
                )
            if t + 1 < TC:
                xc, x8 = xc_next, x8_next

    nc.compile()
    return nc


def _get_nc():
    if "nc" not in _CACHE:
        _CACHE["nc"] = _build()
    return _CACHE["nc"]


def _install_trace_shim():
    """This image's antenv lacks axon_hooks; register the NTFF profile hook
    ourselves so run_bass_kernel_spmd(trace=True) can capture exec_time_ns."""
    import sys
    import types

    if "antenv.axon_hooks" in sys.modules:
        return
    import antenv

    mod = types.ModuleType("antenv.axon_hooks")
    state = {"hook": None}
    mod.set_axon_ntff_profile_hook = lambda h: state.__setitem__("hook", h)
    mod.get_axon_ntff_profile_hook = lambda: state["hook"]
    sys.modules["antenv.axon_hooks"] = mod
    antenv.axon_hooks = mod

    from trn_agent_boot.trn_boot import _ntff_profile_via_ctypes

    mod.set_axon_ntff_profile_hook(
        _ntff_profile_via_ctypes("/opt/axon/libaxon_pjrt.so")
    )

    # No S3 in this container; keep artifacts local.
    import concourse.bass_utils as bu

    bu.upload_artifacts = lambda tmpdir: f"local://{tmpdir}"


def kernel(x, W, b, A_all, B_all, lora_idx, _trace=False):
    global LAST_EXEC_TIME_NS
    from concourse.bass_utils import run_bass_kernel_spmd

    if _trace:
        try:
            _install_trace_shim()
        except Exception as e:  # degrade to untraced run
            print(f"trace shim failed ({e!r}); running untraced")
            _trace = False

    x = np.asarray(x, dtype=np.float32)
    W = np.asarray(W, dtype=np.float32)
    b = np.asarray(b, dtype=np.float32)
    A_all = np.asarray(A_all, dtype=np.float32)
    B_all = np.asarray(B_all, dtype=np.float32)
    lora_idx = np.asarray(lora_idx, dtype=np.int32)

    # Host-side weight reformat (replicated across cores), partition-major.
    # wT[g, p, k, o] = W[g*WGC+o, k*P+p] for the KB bf16 k-tiles.
    wT_np = np.ascontiguousarray(
        W.astype(_BF16).reshape(WG, WGC, KT, P)[:, :, :KB].transpose(0, 3, 2, 1)
    )
    # wF[p, k, o] = e4m3(SW * W[o, (KB+k)*P+p])
    wF_np = np.ascontiguousarray(
        (W[:, KB * P :] * SW).astype(_F8).reshape(D_OUT, KF, P).transpose(2, 1, 0)
    )
    # aT[p, k, c] = A_flat[c, k*P+p] (bf16 k-tiles); aF: fp8 tail tiles
    A_kt = A_all.reshape(LR, KT, P)
    aT_np = np.ascontiguousarray(A_kt[:, :KB].astype(_BF16).transpose(2, 1, 0))
    aF_np = np.ascontiguousarray(
        (A_kt[:, KB:] * SW).astype(_F8).transpose(2, 1, 0)
    )
    # bF[c, o] = B_all[c//R, o, c%R]
    bF_np = np.ascontiguousarray(B_all.transpose(0, 2, 1)).reshape(LR, D_OUT).astype(
        _BF16
    )
    bias_np = np.ascontiguousarray(b.reshape(OI, P).T).astype(np.float32)

    xb = (x * PSCALE).astype(_BF16)  # exact power-of-2 scale
    x8 = (x[:, KB * P :] * SX).astype(_F8)
    adapters = (np.arange(LR, dtype=np.int32) // R)[:, None]  # [LR, 1]

    in_maps = []
    for i in range(M_CORES):
        s = slice(i * NS, (i + 1) * NS)
        # xT[t, p, k, j] = PSCALE * x[i*NS + t*TW + j, k*P + p], k < KB
        xT_i = np.ascontiguousarray(
            xb[s].reshape(TC, TW, KT, P)[:, :, :KB].transpose(0, 3, 2, 1)
        )
        xF_i = np.ascontiguousarray(
            x8[s].reshape(TC, TW, KF, P).transpose(0, 3, 2, 1)
        )
        idx = lora_idx[s]
        mfull = (adapters == idx[None, :]).astype(np.float32) * SCALE  # [LR, NS]
        mT_i = np.ascontiguousarray(
            mfull.astype(_BF16).reshape(LR, TC, TW).transpose(1, 0, 2)
        )
        in_maps.append(
            {
                "xT": xT_i,
                "xF": xF_i,
                "wT": wT_np,
                "wF": wF_np,
                "aT": aT_np,
                "aF": aF_np,
                "bF": bF_np,
                "bias": bias_np,
                "mT": mT_i,
            }
        )

    nc = _get_nc()
    res = run_bass_kernel_spmd(
        nc, in_maps, core_ids=list(range(M_CORES)), trace=_trace
    )
    LAST_EXEC_TIME_NS = res.exec_time_ns

    out = np.empty((N, D_OUT), dtype=np.float32)
    for i in range(M_CORES):
        out[i * NS : (i + 1) * NS] = res.results[i]["outT"].T
    return out


# revision 59
# speedup vs baseline: 1.0020x; 1.0015x over previous
"""Fused multi-LoRA linear layer on 8 TRN2 NeuronCores.

out = x @ W.T + b + scale * mask(x @ A_all^T) @ B_flat

Sharding: data-parallel over the token dim N (32768 -> 8 x 4096).
Weights (W, A_all, B_all, b) are replicated; each core computes its token
shard fully, so no collectives are needed.

Device-side layout: the kernel computes out^T [d_out, tokens] so that the
bias is a per-partition scalar (fused into the PSUM->SBUF eviction on the
Scalar engine) and neither x nor the output needs an on-chip transpose.
All streamed inputs are laid out partition-major on the host so every DMA
is a contiguous-per-partition block transfer.

Mixed precision: the last KF=6 of 16 k-tiles of the main matmul run in
fp8e4 (e4m3) with DoubleRow perf mode (2 contraction rows per PE
cell-cycle); the other 10 k-tiles and the LoRA path stay bf16.
fp8 operands are pre-scaled on the host (x*16, W*256) so their PSUM
contribution lands at 4096x; the bf16 x tiles are pre-scaled by 4096
(exact, power of two) so the whole PSUM accumulates at 4096x, and the
eviction folds the 2^-12 rescale into the scalar-engine activation.
Measured rel err vs the f32 reference: 1.945e-2 (gate: 2e-2).
"""

import numpy as np
import ml_dtypes

# Problem constants (hardcoded per harness contract).
N, D_IN, D_OUT, L, R = 32768, 2048, 2048, 8, 16
SCALE = 32.0 / 16.0
M_CORES = 8
NS = N // M_CORES  # 4096 tokens per core
P = 128
KT = D_IN // P  # 16 k-tiles
KF = 6  # fp8 k-tiles of the main matmul (the last KF)
KB = KT - KF  # bf16 k-tiles
OI = D_OUT // P  # 16 output row-chunks of 128
TW = 512  # token tile width (moving free dim)
TC = NS // TW  # 8 token chunks per core
LR = L * R  # 128
WG = 4  # W column groups (bf16 part)
WGC = D_OUT // WG  # 512 columns per group
SX = 16.0  # fp8 x scale
SW = 256.0  # fp8 W / A scale
PSCALE = SX * SW  # 4096: PSUM runs at this scale

_BF16 = ml_dtypes.bfloat16
_F8 = ml_dtypes.float8_e4m3

_CACHE = {}

LAST_EXEC_TIME_NS = None


def _build():
    import concourse.bass as bass  # noqa: F401
    import concourse.tile as tile
    from concourse import bacc, mybir
    from contextlib import ExitStack

    bf16 = mybir.dt.bfloat16
    f8 = mybir.dt.float8e4
    f32 = mybir.dt.float32
    DR = mybir.MatmulPerfMode.DoubleRow

    nc = bacc.Bacc(
        "TRN2",
        target_bir_lowering=False,
        debug=False,
        num_devices=M_CORES,
    )

    # Host-prepared, partition-major layouts (see kernel()):
    #   xT  [TC, P, KB, TW] : xT[t, p, k, j] = 4096*x[t*TW+j, k*P+p]   (bf16)
    #   xF  [TC, P, KF, TW] : xF[t, p, k, j] = e4m3(16*x[t*TW+j, (KB+k)*P+p])
    #   wT  [WG, P, KB, WGC]: wT[g, p, k, o] = W[g*WGC+o, k*P+p]       (bf16)
    #   wF  [P, KF, D_OUT]  : wF[p, k, o] = e4m3(256*W[o, (KB+k)*P+p])
    #   aT  [P, KB, LR]     : aT[p, k, c] = A_flat[c, k*P+p]           (bf16)
    #   aF  [P, KF, LR]     : aF[p, k, c] = e4m3(256*A_flat[c, (KB+k)*P+p])
    #   bF  [P, D_OUT]      : bF[c, o] = B_all[c//R, o, c%R]           (bf16)
    #   mT  [TC, P, TW]     : one-hot adapter mask * SCALE             (bf16)
    #   bias [P, OI]        : bias[p, oi] = b[oi*P+p]                  (f32)
    xT = nc.dram_tensor("xT", [TC, P, KB, TW], bf16, kind="ExternalInput").ap()
    xF = nc.dram_tensor("xF", [TC, P, KF, TW], f8, kind="ExternalInput").ap()
    wT = nc.dram_tensor("wT", [WG, P, KB, WGC], bf16, kind="ExternalInput").ap()
    wF = nc.dram_tensor("wF", [P, KF, D_OUT], f8, kind="ExternalInput").ap()
    aT = nc.dram_tensor("aT", [P, KB, LR], bf16, kind="ExternalInput").ap()
    aF = nc.dram_tensor("aF", [P, KF, LR], f8, kind="ExternalInput").ap()
    bF = nc.dram_tensor("bF", [P, D_OUT], bf16, kind="ExternalInput").ap()
    bias = nc.dram_tensor("bias", [P, OI], f32, kind="ExternalInput").ap()
    mT = nc.dram_tensor("mT", [TC, P, TW], bf16, kind="ExternalInput").ap()
    outT = nc.dram_tensor("outT", [D_OUT, NS], f32, kind="ExternalOutput").ap()

    from concourse.tile_rust import add_dep_helper

    with tile.TileContext(nc) as tc, ExitStack() as ctx:
        warm_pool = ctx.enter_context(tc.tile_pool(name="warm", bufs=1))
        wt_pool = ctx.enter_context(tc.tile_pool(name="wt", bufs=WG))
        wf_pool = ctx.enter_context(tc.tile_pool(name="wf", bufs=1))
        af_pool = ctx.enter_context(tc.tile_pool(name="af", bufs=1))
        bf_pool = ctx.enter_context(tc.tile_pool(name="bfp", bufs=1))
        bias_pool = ctx.enter_context(tc.tile_pool(name="bias", bufs=1))
        mask_pool = ctx.enter_context(tc.tile_pool(name="mask", bufs=1))
        x_pool = ctx.enter_context(tc.tile_pool(name="x", bufs=2))
        x8_pool = ctx.enter_context(tc.tile_pool(name="x8", bufs=2))
        u_pool = ctx.enter_context(tc.tile_pool(name="u", bufs=2))
        o_pool = ctx.enter_context(tc.tile_pool(name="o", bufs=8))
        pw_pool = ctx.enter_context(tc.tile_pool(name="pw", bufs=1, space="PSUM"))
        pu_pool = ctx.enter_context(tc.tile_pool(name="pu", bufs=2, space="PSUM"))
        po_pool = ctx.enter_context(tc.tile_pool(name="po", bufs=5, space="PSUM"))

        # Short PE warmup: real matmuls are ready almost immediately (the
        # first x quarter lands before the engine preamble finishes), so the
        # warmup only needs to absorb a slice of the HAM cold-clock window.
        warm = warm_pool.tile([P, P], bf16)
        nc.vector.memset(warm[:], 0.0)
        pw = pw_pool.tile([P, P], mybir.dt.float32)
        for _ in range(16):
            nc.tensor.matmul(pw[:], warm[:], warm[:], start=True, stop=True)

        # A tiles ride the front of the scalar ring (idle until the gated W
        # stream starts); the first x chunk is the sync-ring critical path
        # (issued inside the t=0 loop iteration below). mask/bias go on the
        # gpsimd ring - they are not needed until the first up-projection.
        at = af_pool.tile([P, KB, LR], bf16)
        nc.scalar.dma_start(at[:], aT[:, :, :])
        af8 = af_pool.tile([P, KF, LR], f8)
        nc.scalar.dma_start(af8[:], aF[:, :, :])
        bias_t = bias_pool.tile([P, OI], f32)
        nc.gpsimd.dma_start(bias_t[:], bias[:, :])
        mask_t = mask_pool.tile([P, TC, TW], bf16)
        nc.gpsimd.dma_start(mask_t[:], mT.rearrange("t p j -> p t j"))

        # Big W load + B_flat stream on the scalar HWDGE ring, gated behind
        # the first x quarter so the critical-path x DMA is not starved at
        # kickoff. Order: g0 (needed by oi=0), then the small bF/wF (needed
        # by every oi's tail), then g1-g3.
        wts = [
            wt_pool.tile([P, KB, WGC], bf16, name="wt_g") for _ in range(WG)
        ]
        wf_t = wf_pool.tile([P, KF, D_OUT], f8)
        bf_t = bf_pool.tile([P, D_OUT], bf16)
        wg0_dma = nc.scalar.dma_start(wts[0][:], wT[0])
        nc.scalar.dma_start(bf_t[:], bF[:, :])
        for g in range(1, WG):
            nc.scalar.dma_start(wts[g][:], wT[g])

        # Chunk 0 streams in k-tile slices so the down-projection can start
        # on tile 0 before the whole chunk lands; W yields HBM bandwidth
        # until the first two slices are in.
        xc = x_pool.tile([P, KB, TW], bf16, name="xc", bufs=2)
        for lo, hi in ((0, 4), (4, 8), (8, KB)):
            xq_dma = nc.sync.dma_start(xc[:, lo:hi, :], xT[0, :, lo:hi, :])
            if hi <= 8:
                add_dep_helper(
                    wg0_dma.ins, xq_dma.ins, sync=True,
                    reason="critical path first",
                )
        x8 = x8_pool.tile([P, KF, TW], f8)
        nc.sync.dma_start(x8[:], xF[0])
        # wF rides the sync ring behind chunk 0's x (the scalar ring is busy
        # with g0/bF until past the point oi=0 needs it).
        nc.sync.dma_start(wf_t[:], wF[:, :, :])

        for t in range(TC):
            # Prefetch the next chunk's x now, ahead of this chunk's output
            # DMAs on the in-order sync ring - otherwise the next
            # down-projection waits for 4 MB of outs to drain.
            if t + 1 < TC:
                xc_next = x_pool.tile([P, KB, TW], bf16, name="xc", bufs=2)
                nc.sync.dma_start(xc_next[:], xT[t + 1])
                x8_next = x8_pool.tile([P, KF, TW], f8, name="x8", bufs=2)
                nc.sync.dma_start(x8_next[:], xF[t + 1])

            # LoRA down-projection: u^T[c, tok] for all adapters at once,
            # at 4096x scale (x carries the 4096). The last KF k-tiles ride
            # the fp8 DoubleRow path like the main matmul.
            pu = pu_pool.tile([P, TW], mybir.dt.float32)
            for k in range(KB):
                nc.tensor.matmul(
                    pu[:], at[:, k, :], xc[:, k, :], start=(k == 0), stop=False
                )
            for i in range(KF // 2):
                nc.tensor.matmul(
                    pu[:],
                    af8[:, 2 * i : 2 * i + 2, :],
                    x8[:, 2 * i : 2 * i + 2, :],
                    start=False,
                    stop=(i == KF // 2 - 1),
                    perf_mode=DR,
                )
            # Mask-select adapters + apply scale (mask carries the scale;
            # pu and the main PSUM share the 4096x scale so it cancels).
            um = u_pool.tile([P, TW], bf16)
            nc.vector.tensor_tensor(
                um[:], pu[:], mask_t[:, t, :], op=mybir.AluOpType.mult
            )

            for oi in range(OI):
                wt_g = wts[oi // WG]
                loc = (oi % WG) * P
                po = po_pool.tile([P, TW], mybir.dt.float32)
                for k in range(KB):
                    nc.tensor.matmul(
                        po[:],
                        wt_g[:, k, loc : loc + P],
                        xc[:, k, :],
                        start=(k == 0),
                        stop=False,
                    )
                # fp8 DoubleRow k-tiles (the last KF of the contraction).
                for i in range(KF // 2):
                    nc.tensor.matmul(
                        po[:],
                        wf_t[:, 2 * i : 2 * i + 2, oi * P : (oi + 1) * P],
                        x8[:, 2 * i : 2 * i + 2, :],
                        start=False,
                        stop=False,
                        perf_mode=DR,
                    )
                # LoRA up-projection accumulates into the same PSUM bank.
                nc.tensor.matmul(
                    po[:], bf_t[:, oi * P : (oi + 1) * P], um[:], start=False, stop=True
                )
                ot = o_pool.tile([P, TW], mybir.dt.float32)
                # Eviction rescales PSUM (4096x) and adds the bias.
                nc.scalar.activation(
                    ot[:],
                    po[:],
                    mybir.ActivationFunctionType.Identity,
                    bias=bias_t[:, oi : oi + 1],
                    scale=1.0 / PSCALE,
                )
                # Outs ride the scalar ring (idle after the W preload) so
                # the sync ring carries only the x stream.
                nc.scalar.dma_start(
                    outT[oi * P : (oi + 1) * P, t * TW : (t + 1) * TW], ot[:]
                )
            if t + 1 < TC:
                xc, x8 = xc_next, x8_next

    nc.compile()
    return nc


def _get_nc():
    if "nc" not in _CACHE:
        _CACHE["nc"] = _build()
    return _CACHE["nc"]


def _install_trace_shim():
    """This image's antenv lacks axon_hooks; register the NTFF profile hook
    ourselves so run_bass_kernel_spmd(trace=True) can capture exec_time_ns."""
    import sys
    import types

    if "antenv.axon_hooks" in sys.modules:
        return
    import antenv

    mod = types.ModuleType("antenv.axon_hooks")
    state = {"hook": None}
    mod.set_axon_ntff_profile_hook = lambda h: state.__setitem__("hook", h)
    mod.get_axon_ntff_profile_hook = lambda: state["hook"]
    sys.modules["antenv.axon_hooks"] = mod
    antenv.axon_hooks = mod

    from trn_agent_boot.trn_boot import _ntff_profile_via_ctypes

    mod.set_axon_ntff_profile_hook(
        _ntff_profile_via_ctypes("/opt/axon/libaxon_pjrt.so")
    )

    # No S3 in this container; keep artifacts local.
    import concourse.bass_utils as bu

    bu.upload_artifacts = lambda tmpdir: f"local://{tmpdir}"


def kernel(x, W, b, A_all, B_all, lora_idx, _trace=False):
    global LAST_EXEC_TIME_NS
    from concourse.bass_utils import run_bass_kernel_spmd

    if _trace:
        try:
            _install_trace_shim()
        except Exception as e:  # degrade to untraced run
            print(f"trace shim failed ({e!r}); running untraced")
            _trace = False

    x = np.asarray(x, dtype=np.float32)
    W = np.asarray(W, dtype=np.float32)
    b = np.asarray(b, dtype=np.float32)
    A_all = np.asarray(A_all, dtype=np.float32)
    B_all = np.asarray(B_all, dtype=np.float32)
    lora_idx = np.asarray(lora_idx, dtype=np.int32)

    # Host-side weight reformat (replicated across cores), partition-major.
    # wT[g, p, k, o] = W[g*WGC+o, k*P+p] for the KB bf16 k-tiles.
    wT_np = np.ascontiguousarray(
        W.astype(_BF16).reshape(WG, WGC, KT, P)[:, :, :KB].transpose(0, 3, 2, 1)
    )
    # wF[p, k, o] = e4m3(SW * W[o, (KB+k)*P+p])
    wF_np = np.ascontiguousarray(
        (W[:, KB * P :] * SW).astype(_F8).reshape(D_OUT, KF, P).transpose(2, 1, 0)
    )
    # aT[p, k, c] = A_flat[c, k*P+p] (bf16 k-tiles); aF: fp8 tail tiles
    A_kt = A_all.reshape(LR, KT, P)
    aT_np = np.ascontiguousarray(A_kt[:, :KB].astype(_BF16).transpose(2, 1, 0))
    aF_np = np.ascontiguousarray(
        (A_kt[:, KB:] * SW).astype(_F8).transpose(2, 1, 0)
    )
    # bF[c, o] = B_all[c//R, o, c%R]
    bF_np = np.ascontiguousarray(B_all.transpose(0, 2, 1)).reshape(LR, D_OUT).astype(
        _BF16
    )
    bias_np = np.ascontiguousarray(b.reshape(OI, P).T).astype(np.float32)

    xb = (x * PSCALE).astype(_BF16)  # exact power-of-2 scale
    x8 = (x[:, KB * P :] * SX).astype(_F8)
    adapters = (np.arange(LR, dtype=np.int32) // R)[:, None]  # [LR, 1]

    in_maps = []
    for i in range(M_CORES):
        s = slice(i * NS, (i + 1) * NS)
        # xT[t, p, k, j] = PSCALE * x[i*NS + t*TW + j, k*P + p], k < KB
        xT_i = np.ascontiguousarray(
            xb[s].reshape(TC, TW, KT, P)[:, :, :KB].transpose(0, 3, 2, 1)
        )
        xF_i = np.ascontiguousarray(
            x8[s].reshape(TC, TW, KF, P).transpose(0, 3, 2, 1)
        )
        idx = lora_idx[s]
        mfull = (adapters == idx[None, :]).astype(np.float32) * SCALE  # [LR, NS]
        mT_i = np.ascontiguousarray(
            mfull.astype(_BF16).reshape(LR, TC, TW).transpose(1, 0, 2)
        )
        in_maps.append(
            {
                "xT": xT_i,
                "xF": xF_i,
                "wT": wT_np,
                "wF": wF_np,
                "aT": aT_np,
                "aF": aF_np,
                "bF": bF_np,
                "bias": bias_np,
                "mT": mT_i,
            }
        )

    nc = _get_nc()
    res = run_bass_kernel_spmd(
        nc, in_maps, core_ids=list(range(M_CORES)), trace=_trace
    )
    LAST_EXEC_TIME_NS = res.exec_time_ns

    out = np.empty((N, D_OUT), dtype=np.float32)
    for i in range(M_CORES):
        out[i * NS : (i + 1) * NS] = res.results[i]["outT"].T
    return out
